# revision 25
# baseline (speedup 1.0000x reference)
"""Expert-parallel MoE FFN kernel for Trainium2 (8 NeuronCores).

Strategy: each of the 8 experts lives on its own core. Rows are routed
host-side (argsort by note_type_pos), padded to a uniform capacity C,
and shipped feature-major (transposed) so the device kernel is a pure
dense 2-layer MLP with the feature dimension on SBUF partitions:

    hT = relu(W1.T @ xT + b1)     [F, C]
    yT = W2.T @ hT + b2           [H, C]

x and the weights are shipped in bf16 (error ~3e-3 vs the 2e-2 gate;
half the HBM traffic of f32); biases, PSUM accumulation and the output
stay f32. The body is PE-bound at the bf16 roofline (512*C cycles), so
the remaining time lives at the two ends:

Startup: descriptor issue is ~600ns serialized on the DGE path and the
first matmul can't start before its x/W chunks land, so the first
descriptors are small k-pair slices of xT and W1 interleaved in exact
consumption order (the first L1 sweep walks k-pairs across fc0-3), and
Pool-memset-fed warmup matmuls keep the PE's p-state ramp running so
real matmuls issue warm the moment data arrives.

Drain: each DMA pays a ~625ns sem-gated desc-gen (serialized globally),
~650ns DGE latency, and a ~900ns completion delay before the end
barrier sees it. The final tile's writebacks taper (m0-3 at m==3,
m4-5 at m==5), and the last two m-groups run as a column-chunked
finale (224 cols, then the remainder) where each chunk's writeback is
ONE descriptor spanning both m-rows — two pipelined desc-gens instead
of three stacked ones, with a small final transfer.
"""

import sys

sys.path.insert(0, "/opt/trn_rl_repo")

import numpy as np

import concourse.bass as bass
import concourse.mybir as mybir
from concourse import bacc
from concourse.tile import TileContext

H = 1024
F = 4096
N_EXPERTS = 8
P = 128
KH = H // P   # 8
KF = F // P   # 32
FB = 1024     # F block size (weights streamed per block)
NFB = F // FB  # 4
FC = FB // P  # 8


def _row_tiles(C):
    """Split C columns into equal chunks <=512 (multiples of 16)."""
    n = -(-C // 512)
    rw = -(-C // n)
    rw = ((rw + 15) // 16) * 16
    tiles = []
    s = 0
    while s < C:
        w = min(rw, C - s)
        tiles.append((s, w))
        s += w
    return tiles


def build_expert_kernel(C, reps=1, n_wu=6, fin_a=232):
    """One expert's 2-layer MLP: xT [H, C] -> yT [H, C].

    w1 arrives host-packed as [P, NFB*FC*KH*128] (partition-major blocked:
    per partition p, element (fb, fc, k, j) = W1[k*128+p, fb*512+fc*128+j])
    so every load slice is contiguous per partition — no sub-512B runs.
    """
    f32 = mybir.dt.float32
    bf16 = mybir.dt.bfloat16
    nc = bacc.Bacc(None, target_bir_lowering=False)
    xT = nc.dram_tensor("xT", [H, C], bf16, kind="ExternalInput")
    w1 = nc.dram_tensor("w1", [P, NFB * FC * KH * P], bf16,
                        kind="ExternalInput")
    b1v = nc.dram_tensor("b1v", [P, KF], f32, kind="ExternalInput")
    w2 = nc.dram_tensor("w2", [F, H], bf16, kind="ExternalInput")
    b2v = nc.dram_tensor("b2v", [P, KH], f32, kind="ExternalInput")
    yT = nc.dram_tensor("yT", [H, C], bf16, kind="ExternalOutput")

    # partition-major views: one DMA descriptor per logical block
    w1r = w1.rearrange("p (fb fc kj) -> p fb fc kj", fb=NFB, fc=FC)
    w2r = w2.rearrange("(f p) h -> p f h", p=P)   # [P, KF, H]
    xTr = xT.rearrange("(k p) c -> p k c", p=P)   # [P, KH, C]
    yTr = yT.rearrange("(m p) c -> p m c", p=P)   # [P, KH, C]

    KHP = KH * P  # 1024 packed elements per fc group

    tiles = _row_tiles(C)
    rwmax = max(t[1] for t in tiles)

    with TileContext(nc) as tc:
        with (
            tc.tile_pool(name="consts", bufs=1) as consts,
            tc.tile_pool(name="xp", bufs=1) as xp,
            tc.tile_pool(name="yaccp", bufs=1) as yaccp,
            tc.tile_pool(name="youtp", bufs=1) as youtp,
            tc.tile_pool(name="w1p", bufs=2) as w1p,
            tc.tile_pool(name="w2p", bufs=2) as w2p,
            tc.tile_pool(name="hp", bufs=3) as hp,
            tc.tile_pool(name="psh", bufs=4, space="PSUM") as psh,
            tc.tile_pool(name="psy", bufs=4, space="PSUM") as psy,
        ):
            # Startup, in consumption order: the first L1 sweep walks
            # k-pairs kq across fc0-3, so ship xT/w1 k-pair slices
            # alternating; each chunk lands just ahead of the matmuls
            # that consume it, and the first matmul can go at ~3.3us.
            r0_, rw_ = tiles[0]
            w1blk0 = w1p.tile([P, FC, KHP], bf16, tag="w1blk")
            xT_sb = xp.tile([P, KH, C], bf16, tag="xT")
            # w chunk leads its x partner: the first matmul needs both,
            # and the bigger transfer first minimizes the ready time of
            # the pair (data serializes on the DMA engines)
            nc.sync.dma_start(w1blk0[:, 0:4, 0:2 * P],
                              w1r[:, 0, 0:4, 0:2 * P])
            nc.sync.dma_start(xT_sb[:, 0:2, r0_:r0_ + rw_],
                              xTr[:, 0:2, r0_:r0_ + rw_])
            nc.sync.dma_start(w1blk0[:, 0:4, 2 * P:4 * P],
                              w1r[:, 0, 0:4, 2 * P:4 * P])
            nc.sync.dma_start(xT_sb[:, 2:4, r0_:r0_ + rw_],
                              xTr[:, 2:4, r0_:r0_ + rw_])
            nc.sync.dma_start(w1blk0[:, 0:4, 4 * P:6 * P],
                              w1r[:, 0, 0:4, 4 * P:6 * P])
            nc.sync.dma_start(xT_sb[:, 4:6, r0_:r0_ + rw_],
                              xTr[:, 4:6, r0_:r0_ + rw_])
            nc.sync.dma_start(w1blk0[:, 0:4, 6 * P:8 * P],
                              w1r[:, 0, 0:4, 6 * P:8 * P])
            nc.sync.dma_start(xT_sb[:, 6:8, r0_:r0_ + rw_],
                              xTr[:, 6:8, r0_:r0_ + rw_])
            # b1 before the fc4-7 weights: the first activation needs it,
            # and that act's completion frees the psum bank fc4 reuses
            b1_sb = consts.tile([P, KF], f32, tag="b1")
            nc.sync.dma_start(b1_sb[:], b1v[:, :])
            # fc4-7 stream singly so each group's weights land just ahead
            # of its matmuls (finer splits cost a ~650ns desc-gen slot
            # that squeezes the later chunks — net loss)
            nc.sync.dma_start(w1blk0[:, 4:5, :], w1r[:, 0, 4:5, :])
            nc.sync.dma_start(w1blk0[:, 5:6, :], w1r[:, 0, 5:6, :])
            nc.sync.dma_start(w1blk0[:, 6:7, :], w1r[:, 0, 6:7, :])
            nc.sync.dma_start(w1blk0[:, 7:8, :], w1r[:, 0, 7:8, :])
            # w2 fb0 streams in m-pair slices against the L2 m-loop pace
            w2blk0 = w2p.tile([P, FC, H], bf16, tag="w2blk")
            nc.sync.dma_start(w2blk0[:, :, 0:2 * P], w2r[:, 0:FC, 0:2 * P])
            b2_sb = consts.tile([P, KH], f32, tag="b2")
            nc.sync.dma_start(b2_sb[:], b2v[:, :])
            for m2 in range(1, KH // 2):
                nc.sync.dma_start(
                    w2blk0[:, :, 2 * m2 * P:2 * (m2 + 1) * P],
                    w2r[:, 0:FC, 2 * m2 * P:2 * (m2 + 1) * P])
            for (r0, rw) in tiles[1:]:
                nc.sync.dma_start(xT_sb[:, :, r0:r0 + rw],
                                  xTr[:, :, r0:r0 + rw])

            def load_w1(fb):
                w1blk = w1p.tile([P, FC, KHP], bf16, tag="w1blk")
                nc.sync.dma_start(w1blk[:, :, :], w1r[:, fb])
                return w1blk

            def load_w2(fb):
                w2blk = w2p.tile([P, FC, H], bf16, tag="w2blk")
                nc.sync.dma_start(w2blk[:, :, :],
                                  w2r[:, fb * FC:(fb + 1) * FC, :])
                return w2blk

            # Warmup: Pool-engine memset feeds the PE early so the chained
            # warmup matmuls hold the p-state ramp until the first data
            # chunks land; Relu touch preloads the act table (~1.3us)
            # inside the DMA window.
            wu = consts.tile([P, 512], bf16, tag="wu")
            nc.gpsimd.memset(wu[:], 0.0)
            if n_wu:
                wups = psy.tile([P, 512], f32, tag="py", name="wups")
                for i in range(n_wu):
                    nc.tensor.matmul(wups[:], wu[:, 0:P], wu[:],
                                     start=(i == 0), stop=(i == n_wu - 1))
            wuact = consts.tile([P, 16], f32, tag="wuact")
            nc.scalar.activation(wuact[:], wu[:, 0:16],
                                 mybir.ActivationFunctionType.Relu)

            yacc = yaccp.tile([P, KH, C], f32, tag="yacc")
            yout = youtp.tile([P, KH, C], bf16, tag="yout")

            def layer1(fb, w1blk, r0, rw, kpair_major=False):
                h_sb = hp.tile([P, FC, rwmax], bf16, tag="h")
                if kpair_major:
                    # startup path: walk k-pairs across fc0-3 (all psh
                    # banks) so the PE starts on the first k-pair chunks
                    # while later pairs are still in flight
                    nfc = min(FC, 4)
                    phs = []
                    for fci in range(nfc):
                        ph_i = psh.tile([P, rw], f32, tag="ph",
                                        name=f"ph_s{fci}")
                        phs.append(ph_i)
                    for kq in range(KH // 2):
                        for fc in range(nfc):
                            for k in (2 * kq, 2 * kq + 1):
                                nc.tensor.matmul(
                                    phs[fc][:],
                                    w1blk[:, fc, k * P:(k + 1) * P],
                                    xT_sb[:, k, r0:r0 + rw],
                                    start=(k == 0), stop=(k == KH - 1))
                            if kq == KH // 2 - 1:
                                # act inline so the psh bank frees for
                                # the fc4-7 groups right behind
                                nc.scalar.activation(
                                    h_sb[:, fc, :rw], phs[fc][:],
                                    mybir.ActivationFunctionType.Relu,
                                    bias=b1_sb[:,
                                               fb * FC + fc:fb * FC + fc + 1])
                else:
                    nfc = 0
                for fc in range(nfc, FC):
                    ph = psh.tile([P, rw], f32, tag="ph")
                    for k in range(KH):
                        nc.tensor.matmul(
                            ph[:],
                            w1blk[:, fc, k * P:(k + 1) * P],
                            xT_sb[:, k, r0:r0 + rw],
                            start=(k == 0), stop=(k == KH - 1))
                    nc.scalar.activation(
                        h_sb[:, fc, :rw], ph[:],
                        mybir.ActivationFunctionType.Relu,
                        bias=b1_sb[:, fb * FC + fc:fb * FC + fc + 1])
                return h_sb

            def layer2(fb, w2blk, h_sb, m, r0, rw, last, final_tile):
                taper = final_tile and last and fb == NFB - 1
                if taper and m == KH - 2 and rw >= fin_a + 48:
                    # m6+m7 run as one column-chunked finale (emitted at
                    # m==7); nothing to do at m==6
                    return
                if taper and m == KH - 1 and rw >= fin_a + 48:
                    # Finale: the last two m-groups run per column chunk
                    # (fin_a then the remainder) so each chunk's
                    # writeback is ONE descriptor spanning both rows.
                    # Desc-gens are sem-gated AND serialized at ~625ns:
                    # the second chunk is sized so its matmul+add chain
                    # outlasts the first chunk's gen (gens pipeline).
                    for (c0, cw) in ((0, fin_a), (fin_a, rw - fin_a)):
                        for mm in (KH - 2, KH - 1):
                            py = psy.tile([P, cw], f32, tag="py")
                            for fc in range(FC):
                                nc.tensor.matmul(
                                    py[:],
                                    w2blk[:, fc, mm * P:(mm + 1) * P],
                                    h_sb[:, fc, c0:c0 + cw],
                                    start=(fc == 0), stop=(fc == FC - 1))
                            nc.vector.tensor_add(
                                out=yout[:, mm, r0 + c0:r0 + c0 + cw],
                                in0=yacc[:, mm, r0 + c0:r0 + c0 + cw],
                                in1=py[:])
                        eng = nc.scalar if c0 else nc.sync
                        eng.dma_start(
                            yTr[:, KH - 2:KH, r0 + c0:r0 + c0 + cw],
                            yout[:, KH - 2:KH, r0 + c0:r0 + c0 + cw])
                    return
                py = psy.tile([P, rw], f32, tag="py")
                for fc in range(FC):
                    nc.tensor.matmul(
                        py[:],
                        w2blk[:, fc, m * P:(m + 1) * P],
                        h_sb[:, fc, :rw],
                        start=(fc == 0), stop=(fc == FC - 1))
                if fb == 0:
                    # fold the layer-2 bias into the first partial
                    nc.scalar.activation(
                        yacc[:, m, r0:r0 + rw], py[:],
                        mybir.ActivationFunctionType.Identity,
                        bias=b2_sb[:, m:m + 1])
                elif fb < NFB - 1:
                    nc.vector.tensor_add(
                        out=yacc[:, m, r0:r0 + rw],
                        in0=yacc[:, m, r0:r0 + rw], in1=py[:])
                else:
                    # final accumulation narrows to the bf16 output stage
                    nc.vector.tensor_add(
                        out=yout[:, m, r0:r0 + rw],
                        in0=yacc[:, m, r0:r0 + rw], in1=py[:])
                    if not last:
                        return
                    # stream writebacks behind the adds; the final tile
                    # tapers (m0-3, m4-5, m6, then the m7 split) so the
                    # trailing descriptors shrink toward the end
                    if taper and m in (3, 5):
                        m0 = {3: 0, 5: 4}[m]
                        nc.sync.dma_start(
                            yTr[:, m0:m + 1, r0:r0 + rw],
                            yout[:, m0:m + 1, r0:r0 + rw])
                    elif taper and m == KH - 1:
                        # narrow final tile: finale didn't apply, write
                        # m6-7 here
                        nc.sync.dma_start(
                            yTr[:, KH - 2:KH, r0:r0 + rw],
                            yout[:, KH - 2:KH, r0:r0 + rw])
                    elif not taper and m in (KH // 2 - 1, KH - 1):
                        m0 = 0 if m == KH // 2 - 1 else KH // 2
                        nc.sync.dma_start(
                            yTr[:, m0:m + 1, r0:r0 + rw],
                            yout[:, m0:m + 1, r0:r0 + rw])

            def body(first_blks=None, last=True):
                for fb in range(NFB):
                    if fb == 0 and first_blks is not None:
                        w1blk, w2blk = first_blks
                    else:
                        w1blk = load_w1(fb)
                        w2blk = load_w2(fb)
                    for ti, (r0, rw) in enumerate(tiles):
                        h_sb = layer1(fb, w1blk, r0, rw,
                                      kpair_major=(fb == 0 and ti == 0
                                                   and first_blks
                                                   is not None))
                        for m in range(KH):
                            layer2(fb, w2blk, h_sb, m, r0, rw, last,
                                   ti == len(tiles) - 1)

            first_blks = (w1blk0, w2blk0)
            for i in range(reps - 1):
                body(first_blks if i == 0 else None, last=False)
            body(first_blks if reps == 1 else None, last=True)
    nc.finalize()
    return nc


# SBUF residency (xT bf16 + yacc f32 at 48*C B/partition) caps capacity.
MAX_C = 1536


def _prepare(x, note_type_pos, W1, b1, W2, b2, cap):
    """Host-side routing: sort rows by expert, pad to capacity C (<= cap)."""
    import ml_dtypes
    bf16 = ml_dtypes.bfloat16
    ntp = np.asarray(note_type_pos).astype(np.int64)
    x = np.ascontiguousarray(np.asarray(x, dtype=np.float32))
    counts = np.bincount(ntp, minlength=N_EXPERTS)
    C = min(int(counts.max()), cap)
    C = max(16, ((C + 15) // 16) * 16)  # 16-aligned, no extra row-tile padding

    order = np.argsort(ntp, kind="stable")
    weights = []
    for e in range(N_EXPERTS):
        # pack W1 partition-major blocked: [P, (fb, fc, k, j)] so device
        # loads are contiguous per partition at any chunking granularity
        w1p_ = (np.asarray(W1[e]).astype(bf16)
                .reshape(KH, P, NFB, FC, P)
                .transpose(1, 2, 3, 0, 4)
                .reshape(P, NFB * FC * KH * P))
        weights.append({
            "w1": np.ascontiguousarray(w1p_),
            "b1v": np.ascontiguousarray(
                np.asarray(b1[e], dtype=np.float32).reshape(KF, P).T),
            "w2": np.ascontiguousarray(np.asarray(W2[e]).astype(bf16)),
            "b2v": np.ascontiguousarray(
                np.asarray(b2[e], dtype=np.float32).reshape(KH, P).T),
        })
    # chunk each expert's rows into groups of <= C; one SPMD launch per group
    launches = []
    off = 0
    expert_rows = []
    for e in range(N_EXPERTS):
        expert_rows.append(order[off:off + counts[e]])
        off += counts[e]
    n_launch = max(1, -(-int(counts.max()) // C))
    for g in range(n_launch):
        in_maps, row_idx = [], []
        for e in range(N_EXPERTS):
            rows = expert_rows[e][g * C:(g + 1) * C]
            row_idx.append(rows)
            xe = np.zeros((C, H), dtype=np.float32)
            if len(rows):
                xe[:len(rows)] = x[rows]
            in_maps.append({"xT": np.ascontiguousarray(xe.T.astype(bf16)),
                            **weights[e]})
        launches.append((in_maps, row_idx))
    return launches, C


def kernel(x, note_type_pos, W1, b1, W2, b2):
    launches, C = _prepare(x, note_type_pos, W1, b1, W2, b2, cap=MAX_C)
    nc = build_expert_kernel(C)
    from concourse.bass_utils import run_bass_kernel_spmd
    T = np.asarray(x).shape[0]
    out = np.zeros((T, H), dtype=np.float32)
    for in_maps, row_idx in launches:
        res = run_bass_kernel_spmd(nc, in_maps, core_ids=list(range(N_EXPERTS)))
        for e in range(N_EXPERTS):
            rows = row_idx[e]
            if len(rows):
                out[rows] = res.results[e]["yT"].T[:len(rows)].astype(
                    np.float32)
    return out
